# revision 1
# baseline (speedup 1.0000x reference)
"""AdaBP decoder (gnn_message_passing) Trainium2 Bass kernel, 8 NeuronCores.

Sharding: data-parallel over batch B=1024 across 8 cores (128 samples/core,
matching the 128 SBUF partitions); adapter weights and edge-index tables are
replicated (per the problem's sharding hint). No collectives needed.

On-core layout "Q": edge tensors are partition-minor over a chosen edge
enumeration j -> (partition j%128, free block j//128) with the 128 batch
samples innermost:
  col order jA: edge (v,c) -> (v//128)*dc*128 + c*128 + v%128
      -> view [128, VH, dc, 128]: per-variable (leave-one-out) sums are
         strided adds along the free axis.
  row order jR: edge (m,r) -> (m//128)*dr*128 + r*128 + m%128
      -> view [128, MH, dr, 128]: per-check products are strided ops.
The col<->row edge permutation each BP iteration stages the edge tensor to
HBM as bf16 [E,128] rows and pulls it back with gpsimd.dma_gather (256B row
per edge; output layout is exactly partition-minor). Gather index tables are
precomputed on the host from col_idx/row_idx (int16, 16-partition-wrapped,
replicated across the 8 GPSIMD Q7 cores).

h_step avoids log-domain leave-one-out sums and division entirely:
  t = clip(tanh(lam/2), +-tanh(LLR_CLIP/2))   (signed; signs fold through)
  per-check LOO products via a product tree (12 small mults per group of 6)
  2*atanh(q') = ln(1+q') - ln(1-q')           (two ACT Ln ops)
The reference's lower clamp on |tanh| only matters for division/log forms;
in the product form its effect is <= ~1e-6 absolute and is dropped.

Engine assignment (measured per-op costs on HW: DVE TT 3.3us, ACT 3.8us,
Pool TT 8.3us for 24-block chunks): DVE does clips/products/damp mixes,
ACT does tanh/ln/bf16 casts (ln and tanh alternate act-table sets per half
to bound table reloads), Pool does the gathers (two SWDGE queues, one per
buffer parity) plus two light ops (h2 subtract, one damp mult). Chunked
2-deep (parity) software pipeline; every same-engine dependent op carries an
explicit semaphore wait (engines are deeply pipelined; CoreSim's race
detector validates the schedule). The adapter MLP runs on the PE in bf16
(f32 PSUM accumulation) with W1^T streamed as bf16 half-tiles.

Measured (axon TRN2, slope method T=30 vs T=10 NEFFs, 16-24 back-to-back
executes to amortize ~8-10ms/call dispatch overhead): ~430us per BP
iteration, ~4.5ms total on-device (10 iterations + ~150us adapter).
Full-size relative error vs the jax reference: 4.1e-04 (gate: 2e-2).
"""
import math
import sys

sys.path.insert(0, "/opt/trn_rl_repo")

import numpy as np
from contextlib import ExitStack

from concourse import bass, bacc, mybir

F32 = mybir.dt.float32
BF16 = mybir.dt.bfloat16
I16 = mybir.dt.int16

LLR_CLIP = 15.0
LOG_TANH_LO = -float(np.log(np.tanh(LLR_CLIP / 2)))
T_LO = float(np.tanh(LOG_TANH_LO / 2))
T_HI = float(np.tanh(LLR_CLIP / 2))
EPS = 1e-6
A = mybir.AluOpType
AF = mybir.ActivationFunctionType


class Ctr:
    def __init__(self):
        self.n = 0

    def inc(self, k=1):
        self.n += k
        return self.n


def build_graph(N, M, BS, T, dc=3, dr=6, NCH_ROW=2, NCH_COL=2, n_half=2, skip_adapter=False, adapter_only=False, out_slots=None, fake_gather=False):
    """Build the Bacc graph for one core (SPMD: same graph on all 8 cores)."""
    E = N * dc
    assert M * dr == E and N % 128 == 0 and M % 128 == 0 and BS == 128
    VH, MH, NBLK = N // 128, M // 128, E // 128
    n_hch = n_half * NCH_ROW
    assert NCH_ROW <= 2, "buffer parity depth is 2; gathers may only run 2 chunks ahead"
    assert MH % n_hch == 0
    MH_C = MH // n_hch          # m-groups per H chunk
    HBLK = MH_C * dr            # edge blocks per H chunk
    NHC = n_hch                 # total H chunks
    assert VH % NCH_COL == 0
    VH_C = VH // NCH_COL        # v-groups per V chunk
    CBLK = VH_C * dc            # edge blocks per V chunk

    NT = max(2, N // 512)
    PW = N // NT
    assert PW <= 512
    KT = N // 128
    NT2 = NT // 2
    NWH = N // 2

    nc = bacc.Bacc(num_swdge_queues=2)

    chn_ext = nc.declare_dram_parameter("chn", [128, VH * 128], F32, isOutput=False)
    w1t_ext = nc.declare_dram_parameter("w1t", [N, N], BF16, isOutput=False)
    b1_ext = nc.declare_dram_parameter("b1", [1, N], F32, isOutput=False)
    w2_ext = nc.declare_dram_parameter("w2", [3, N], F32, isOutput=False)
    b2_ext = nc.declare_dram_parameter("b2", [1, 3], F32, isOutput=False)
    ia2r_ext = nc.declare_dram_parameter("idx_a2r", [128, E // 16], I16, isOutput=False)
    ir2a_ext = nc.declare_dram_parameter("idx_r2a", [128, E // 16], I16, isOutput=False)
    OSL = out_slots or T
    out_ext = nc.declare_dram_parameter("out", [OSL, N, 128], F32, isOutput=True)

    v2c_hbm = nc.dram_tensor("v2c_hbm", [E, 128], BF16)  # col (jA) order rows
    h_hbm = nc.dram_tensor("h_hbm", [E, 128], BF16)      # row (jR) order rows
    gam_dram = nc.dram_tensor("gam_dram", [128, 1], F32)

    out_v = out_ext.ap().rearrange("t (vh p) b -> t p vh b", p=128)
    v2c_hbm_v = v2c_hbm.ap().rearrange("(blk p) b -> p blk b", p=128)
    h_hbm_v = h_hbm.ap().rearrange("(blk p) b -> p blk b", p=128)

    es = ExitStack()
    with es:
        block = es.enter_context(nc.Block())
        # DMA semaphores: one per serialized stream so every wait value is
        # deterministic (DMA completions within a queue may reorder).
        sl = es.enter_context(nc.semaphore("sl"))      # input loads (8 DMAs)
        sw = [es.enter_context(nc.semaphore(f"sw{i}")) for i in range(2)]
        sgam = es.enter_context(nc.semaphore("sgam"))  # gamma roundtrip
        svs = [es.enter_context(nc.semaphore(f"svs{i}")) for i in range(2)]
        shs = [es.enter_context(nc.semaphore(f"shs{i}")) for i in range(2)]
        sos = [es.enter_context(nc.semaphore(f"sos{i}")) for i in range(2)]
        sg = [[es.enter_context(nc.semaphore(f"sg{i}_{j}")) for j in range(2)] for i in range(2)]  # [queue][parity]
        sv = es.enter_context(nc.semaphore("sv"))      # DVE (+1)
        sa = es.enter_context(nc.semaphore("sa"))      # ACT (+1)
        sm = es.enter_context(nc.semaphore("sm"))      # PE (+1)
        sp = es.enter_context(nc.semaphore("sp"))      # Pool compute (+1)
        chn_sb = es.enter_context(nc.sbuf_tensor("chn_sb", [128, VH, 128], F32))
        v2c = es.enter_context(nc.sbuf_tensor("v2c", [128, NBLK, 128], BF16))
        c2v = es.enter_context(nc.sbuf_tensor("c2v", [128, NBLK, 128], F32))
        tsH = es.enter_context(nc.sbuf_tensor("tsH", [128, 2 * HBLK, 128], F32))
        sbA = es.enter_context(nc.sbuf_tensor("sbA", [128, max(HBLK, CBLK), 128], F32))
        sbB = es.enter_context(nc.sbuf_tensor("sbB", [128, max(HBLK, CBLK), 128], F32))
        sbC = es.enter_context(nc.sbuf_tensor("sbC", [128, max(HBLK, CBLK), 128], F32))
        t6 = es.enter_context(nc.sbuf_tensor("t6", [128, max(MH_C * dr, 3 * VH_C), 128], F32))
        gbc = es.enter_context(nc.sbuf_tensor("gbc", [128, 128], F32))
        ogbc = es.enter_context(nc.sbuf_tensor("ogbc", [128, 128], F32))
        ia2r = es.enter_context(nc.sbuf_tensor("ia2r", [128, E // 16], I16))
        ir2a = es.enter_context(nc.sbuf_tensor("ir2a", [128, E // 16], I16))
        zacc = es.enter_context(nc.sbuf_tensor("zacc", [128, 3, NT], F32))
        zred = es.enter_context(nc.sbuf_tensor("zred", [128, 4], F32))
        b2bc = es.enter_context(nc.sbuf_tensor("b2bc", [128, 3], F32))
        ps = [es.enter_context(nc.psum_tensor(f"ps{i}", [128, PW], F32)) for i in range(NT)]

        cv, ca, cm, cp = Ctr(), Ctr(), Ctr(), Ctr()
        cw = [Ctr(), Ctr()]
        cvs = [Ctr(), Ctr()]
        cgq = [[Ctr(), Ctr()], [Ctr(), Ctr()]]  # [queue][parity]
        chp = [Ctr(), Ctr()]
        cop = [Ctr(), Ctr()]

        # adapter-phase SBUF aliases
        assert 3 * N <= NBLK * 128
        v2c_flat = v2c.ap().rearrange("p a b -> p (a b)")        # bf16 elems
        c2v_flat = c2v.ap().rearrange("p a b -> p (a b)")
        v2c_f32 = v2c_flat.bitcast(F32)                          # [128, NBLK*64]
        assert N <= NBLK * 64
        b1bc = v2c_f32[:, 0:N]
        chn_bf = v2c_flat[:, 2 * N:3 * N]
        tsH_flat = tsH.ap().rearrange("p a b -> p (a b)")
        assert N <= 2 * HBLK * 128
        h_adapt = tsH_flat[:, 0:N]
        w2bc = [c2v_flat[:, j * N:(j + 1) * N] for j in range(3)]
        sbA_bf = sbA.ap().rearrange("p a b -> p (a b)").bitcast(BF16)
        sbB_bf = sbB.ap().rearrange("p a b -> p (a b)").bitcast(BF16)
        assert NWH <= 2 * 128 * max(HBLK, CBLK)
        wbuf = [sbA_bf[:, 0:NWH], sbB_bf[:, 0:NWH]]
        sbC_flat = sbC.ap().rearrange("p a b -> p (a b)")
        dummy = sbC_flat[:, 0:PW]

        def bf_view(handle, nblk):
            # bf16 view of the first nblk*128 elements (per partition) of a
            # f32 sbuf tensor region
            flat = handle.ap().rearrange("p a b -> p (a b)").bitcast(BF16)
            return flat[:, 0:nblk * 128].rearrange("p (a b) -> p a b", b=128)

        def bf_view_at(handle, f32_blk_off, nblk):
            flat = handle.ap().rearrange("p a b -> p (a b)").bitcast(BF16)
            o = f32_blk_off * 128 * 2
            return flat[:, o:o + nblk * 128].rearrange("p (a b) -> p a b", b=128)

        gbc_b = lambda nb: gbc.ap().unsqueeze(1).broadcast_to([128, nb, 128])
        ogbc_b = lambda nb: ogbc.ap().unsqueeze(1).broadcast_to([128, nb, 128])

        snap = {}

        # ============ sync: input loads + adapter W stream ============
        @block.sync
        def _(sy):
            if skip_adapter:
                sy.dma_start(out=chn_sb[:, :, :],
                             in_=chn_ext.ap().rearrange("p (a b) -> p a b", b=128)
                             ).then_inc(sl, 16)
                sy.dma_start(out=ia2r[:, :], in_=ia2r_ext[:, :]).then_inc(sl, 16)
                sy.dma_start(out=ir2a[:, :], in_=ir2a_ext[:, :]).then_inc(sl, 16)
                return
            sy.dma_start(out=chn_sb[:, :, :],
                         in_=chn_ext.ap().rearrange("p (a b) -> p a b", b=128)
                         ).then_inc(sl, 16)
            sy.dma_start(out=ia2r[:, :], in_=ia2r_ext[:, :]).then_inc(sl, 16)
            sy.dma_start(out=ir2a[:, :], in_=ir2a_ext[:, :]).then_inc(sl, 16)
            sy.dma_start(out=b1bc,
                         in_=bass.AP(b1_ext, 0, [[0, 128], [1, N]])).then_inc(sl, 16)
            for j in range(3):
                sy.dma_start(out=w2bc[j],
                             in_=bass.AP(w2_ext, j * N, [[0, 128], [1, N]])
                             ).then_inc(sl, 16)
            sy.dma_start(out=b2bc[:, :],
                         in_=bass.AP(b2_ext, 0, [[0, 128], [1, 3]])).then_inc(sl, 16)
            for k in range(KT):
                for hf in range(2):
                    i = k * 2 + hf
                    if i >= 2:
                        sy.wait_ge(sm, (i - 1) * NT2)
                    sy.dma_start(
                        out=wbuf[i % 2],
                        in_=w1t_ext[k * 128:(k + 1) * 128, hf * NWH:(hf + 1) * NWH],
                    ).then_inc(sw[i % 2], 16)
                    cw[i % 2].inc(16)

        # ============ ACT: cast chn to bf16 for the matmul ============
        @block.scalar
        def _(s):
            if skip_adapter:
                return
            s.wait_ge(sl, 128)
            s.wait_ge(sa, ca.n)
            s.activation(out=chn_bf.rearrange("p (a b) -> p a b", b=128),
                         in_=chn_sb[:, :, :], func=AF.Copy).then_inc(sa, 1)
            ca.inc(1)
            snap["chn_bf"] = ca.n

        # ============ PE: adapter matmuls ============
        @block.tensor
        def _(pe):
            if skip_adapter:
                snap["mm_all"] = 0
                return
            pe.wait_ge(sl, 128)
            pe.wait_ge(sa, snap["chn_bf"])
            for k in range(KT):
                for hf in range(2):
                    i = k * 2 + hf
                    pe.wait_ge(sw[i % 2], 16 * (i // 2 + 1))
                    for n0 in range(NT2):
                        bank = hf * NT2 + n0
                        pe.matmul(
                            ps[bank][:, :],
                            chn_bf.rearrange("p (a b) -> p a b", b=128)[:, k, :],
                            wbuf[i % 2].rearrange("p (a b) -> p a b", a=NT2)[:, n0, :],
                            start=(k == 0),
                            stop=(k == KT - 1),
                        ).then_inc(sm, 1)
                        cm.inc(1)
            snap["mm_all"] = cm.n

        # ============ DVE: adapter epilogue ============
        @block.vector
        def _(v):
            if skip_adapter:
                return
            v.wait_ge(sm, snap["mm_all"])
            for bank in range(NT):
                v.wait_ge(sv, cv.n)
                v.tensor_tensor(
                    out=h_adapt[:, bank * PW:(bank + 1) * PW],
                    in0=ps[bank][:, :],
                    in1=b1bc[:, bank * PW:(bank + 1) * PW],
                    op=A.add,
                ).then_inc(sv, 1)
                cv.inc(1)
            for j in range(3):
                for bank in range(NT):
                    v.wait_ge(sv, cv.n)
                    v.scalar_tensor_tensor(
                        out=dummy,
                        in0=h_adapt[:, bank * PW:(bank + 1) * PW],
                        scalar=0.0,
                        in1=w2bc[j][:, bank * PW:(bank + 1) * PW],
                        op0=A.max,
                        op1=A.mult,
                        accum_out=zacc[:, j, bank:bank + 1],
                    ).then_inc(sv, 1)
                    cv.inc(1)
            for j in range(3):
                v.wait_ge(sv, cv.n)
                v.tensor_reduce(
                    out=zred[:, j:j + 1], in_=zacc[:, j, :],
                    axis=mybir.AxisListType.X, op=A.add,
                ).then_inc(sv, 1)
                cv.inc(1)
            snap["zred"] = cv.n

        # ============ ACT: gamma ============
        @block.scalar
        def _(s):
            if skip_adapter:
                return
            s.wait_ge(sv, snap["zred"])
            s.wait_ge(sa, ca.n)
            s.activation(out=zred[:, 3:4], in_=zred[:, 0:1],
                         func=AF.Sigmoid, bias=b2bc[:, 0:1]).then_inc(sa, 1)
            ca.inc(1)
            snap["gam_col"] = ca.n

        # ============ sync: gamma roundtrip ============
        @block.sync
        def _(sy):
            if skip_adapter:
                return
            sy.wait_ge(sa, snap["gam_col"])
            sy.dma_start(out=gam_dram[:, :], in_=zred[:, 3:4]).then_inc(sgam, 16)
            sy.wait_ge(sgam, 16)
            sy.dma_start(out=gbc[:, :],
                         in_=bass.AP(gam_dram, 0, [[0, 128], [1, 128]])
                         ).then_inc(sgam, 16)

        # ============ DVE: prologue ============
        @block.vector
        def _(v):
            if skip_adapter:
                v.wait_ge(sl, 48)
                v.memset(gbc[:, :], 0.5).then_inc(sv, 1)
                cv.inc(1)
            else:
                v.wait_ge(sgam, 32)
            v.wait_ge(sv, cv.n)
            v.tensor_scalar(out=ogbc[:, :], in0=gbc[:, :], scalar1=-1.0,
                            scalar2=1.0, op0=A.mult, op1=A.add).then_inc(sv, 1)
            cv.inc(1)
            v.wait_ge(sv, cv.n)
            v.memset(c2v[:, :, :], 0.0).then_inc(sv, 1)
            cv.inc(1)
            v2c_v = v2c.ap().rearrange("p (vh c) b -> p vh c b", c=dc)
            chn_b = chn_sb.ap().unsqueeze(2).broadcast_to([128, VH, dc, 128])
            g_b = gbc.ap().unsqueeze(1).unsqueeze(1).broadcast_to([128, VH, dc, 128])
            v.wait_ge(sv, cv.n)
            v.tensor_tensor(out=v2c_v, in0=chn_b, in1=g_b, op=A.mult).then_inc(sv, 1)
            cv.inc(1)
            snap["prologue_v"] = cv.n

        # ============ sync: stage v2c(1) (state already bf16) ============
        @block.sync
        def _(sy):
            sy.wait_ge(sv, snap["prologue_v"])
            sy.dma_start(out=v2c_hbm_v, in_=v2c[:, :, :]).then_inc(svs[0], 16)
            cvs[0].inc(16)
            snap["v2c_staged", 0] = (cvs[0].n, cvs[1].n)

        # ======================= MAIN LOOP =======================
        # H-chunk buffers: AB = sbA/sbB by parity, TS = tsH slot by parity,
        # Cb = sbC (ACT-only in H), T6 = tree scratch.
        #   gather lam -> AB; tanh: AB -> TS; c1=clip(TS,+-T_HI) -> AB;
        #   tree: AB -> T6; loo: AB,T6 -> TS; num=1+q' -> Cb (ACT);
        #   den=1-q' -> AB (ACT); L1=Ln(Cb) -> TS; L2=Ln(AB) -> Cb;
        #   h2=TS-Cb -> AB (Pool); DMA AB -> h_hbm.
        for t in range(1, (0 if adapter_only else T) + 1):
            # ---------------- H phase ----------------
            for half in range(n_half):
                for cc_ in range(NCH_ROW):
                    c = half * NCH_ROW + cc_
                    q = c % 2

                    @block.gpsimd
                    def _(g, c=c, q=q, t=t):
                        g.wait_ge(svs[0], snap["v2c_staged", t - 1][0])
                        g.wait_ge(svs[1], snap["v2c_staged", t - 1][1])
                        if t == 1 and c < 2:
                            g.wait_ge(sm, snap["mm_all"])
                        if ("hdma", t, c - 2) in snap:
                            g.wait_ge(shs[q], snap["hdma", t, c - 2])
                        if ("war_ab_sv", t, c) in snap:
                            g.wait_ge(sv, snap["war_ab_sv", t, c])
                        if ("war_ab_sp", t, c) in snap:
                            g.wait_ge(sp, snap["war_ab_sp", t, c])
                        dst = bf_view(sbA if q == 0 else sbB, HBLK)
                        if fake_gather:
                            g.dma_start(out=dst,
                                        in_=v2c_hbm.ap().rearrange("(blk p) b -> p blk b", p=128)[:, c * HBLK:(c + 1) * HBLK, :]
                                        ).then_inc(sg[0][q], 16)
                            cgq[0][q].inc(16)
                            snap["gath_fake", t, c] = True
                            snap["gath", t, c] = (cgq[0][q].n, cgq[1][q].n)
                        else:
                            g.dma_gather(
                                out_ap=dst,
                                in_ap=v2c_hbm[:, :],
                                idxs_ap=ia2r[:, c * (HBLK * 128 // 16):(c + 1) * (HBLK * 128 // 16)],
                                num_idxs=HBLK * 128,
                                num_idxs_reg=HBLK * 128,
                                elem_size=128,
                                single_packet=HBLK * 128 <= 1024,
                                queue_num=q,
                            ).then_inc(sg[q][q], 16)
                            cgq[q][q].inc(16)
                        snap["gath", t, c] = (cgq[0][q].n, cgq[1][q].n)

                    @block.scalar
                    def _(s, c=c, q=q, t=t):
                        s.wait_ge(sg[0][q], snap["gath", t, c][0])
                        if ("gath_fake", t, c) not in snap:
                            s.wait_ge(sg[1][q], snap["gath", t, c][1])
                        if ("tsH_war_sv", t, c) in snap:
                            s.wait_ge(sv, snap["tsH_war_sv", t, c])
                        if ("h2", t, c - 2) in snap:
                            s.wait_ge(sp, snap["h2", t, c - 2])
                        src = bf_view(sbA if q == 0 else sbB, HBLK)
                        s.wait_ge(sa, ca.n)
                        s.activation(out=tsH[:, q * HBLK:(q + 1) * HBLK, :],
                                     in_=src, func=AF.Tanh, scale=0.5).then_inc(sa, 1)
                        ca.inc(1)
                        snap["tanh", t, c] = ca.n

                for cc_ in range(NCH_ROW):
                    c = half * NCH_ROW + cc_
                    q = c % 2

                    @block.vector
                    def _(v, c=c, q=q, t=t):
                        v.wait_ge(sa, snap["tanh", t, c])
                        if c == 0:
                            if ("odma", t - 1, NCH_COL - 2) in snap:
                                v.wait_ge(sos[NCH_COL % 2], snap["odma", t - 1, NCH_COL - 2])
                            if ("odma", t - 1, NCH_COL - 1) in snap:
                                v.wait_ge(sos[(NCH_COL - 1) % 2], snap["odma", t - 1, NCH_COL - 1])
                        ts_s = tsH[:, q * HBLK:(q + 1) * HBLK, :]
                        AB = (sbA if q == 0 else sbB)[:, 0:HBLK, :]
                        v.wait_ge(sv, cv.n)
                        v.tensor_scalar(out=AB, in0=ts_s, scalar1=T_HI,
                                        scalar2=-T_HI, op0=A.min, op1=A.max
                                        ).then_inc(sv, 1)
                        cv.inc(1)
                        c1n = cv.n
                        tv = AB.rearrange("p (mh r) b -> p mh r b", r=dr)
                        t6v = t6.ap().rearrange("p (mh r) b -> p mh r b", r=dr)[:, 0:MH_C, :, :]
                        pr = lambda i: tv[:, :, i, :]
                        t6s = lambda i: t6v[:, :, i, :]
                        for (o, a1, a2) in [(0, pr(0), pr(1)), (1, pr(2), pr(3)),
                                            (2, pr(4), pr(5))]:
                            v.wait_ge(sv, c1n)
                            v.tensor_tensor(out=t6s(o), in0=a1, in1=a2, op=A.mult).then_inc(sv, 1)
                            cv.inc(1)
                        efn = cv.n
                        for (o, i1, i2) in [(3, 0, 1), (4, 0, 2), (5, 1, 2)]:
                            v.wait_ge(sv, efn)
                            v.tensor_tensor(out=t6s(o), in0=t6s(i1), in1=t6s(i2), op=A.mult).then_inc(sv, 1)
                            cv.inc(1)
                        cdefn = cv.n
                        lov = ts_s.rearrange("p (mh r) b -> p mh r b", r=dr)
                        lo = lambda i: lov[:, :, i, :]
                        for (o, p_, tt) in [(0, 1, 5), (1, 0, 5), (2, 3, 4),
                                            (3, 2, 4), (4, 5, 3), (5, 4, 3)]:
                            v.wait_ge(sv, cdefn)
                            v.tensor_tensor(out=lo(o), in0=pr(p_), in1=t6s(tt), op=A.mult).then_inc(sv, 1)
                            cv.inc(1)
                        # num -> Cb, den -> AB (Cb WAR: Pool h2 of c-1 read it)
                        Cb = sbC[:, 0:HBLK, :]
                        if ("h2", t, c - 1) in snap:
                            v.wait_ge(sp, snap["h2", t, c - 1])
                        v.wait_ge(sv, cv.n)
                        v.tensor_scalar(out=Cb, in0=ts_s, scalar1=1.0 - EPS,
                                        scalar2=1.0, op0=A.mult, op1=A.add
                                        ).then_inc(sv, 1)
                        cv.inc(1)
                        v.wait_ge(sv, cv.n)
                        v.tensor_scalar(out=AB, in0=ts_s, scalar1=-(1.0 - EPS),
                                        scalar2=1.0, op0=A.mult, op1=A.add
                                        ).then_inc(sv, 1)
                        cv.inc(1)
                        snap["loo", t, c] = cv.n

                    @block.scalar
                    def _(s, c=c, q=q, t=t):
                        s.wait_ge(sv, snap["loo", t, c])
                        AB = (sbA if q == 0 else sbB)[:, 0:HBLK, :]
                        Cb = sbC[:, 0:HBLK, :]
                        ts_s = tsH[:, q * HBLK:(q + 1) * HBLK, :]
                        s.wait_ge(sa, ca.n)
                        s.activation(out=ts_s, in_=Cb, func=AF.Ln).then_inc(sa, 1)
                        ca.inc(1)
                        s.wait_ge(sa, ca.n)
                        s.activation(out=Cb, in_=AB, func=AF.Ln).then_inc(sa, 1)
                        ca.inc(1)
                        snap["lns", t, c] = ca.n

                    @block.gpsimd
                    def _(g, c=c, q=q, t=t):
                        g.wait_ge(sa, snap["lns", t, c])
                        ABh = (sbA if q == 0 else sbB)
                        h2bf = bf_view_at(ABh, 0, HBLK)
                        Cb = sbC[:, 0:HBLK, :]
                        ts_s = tsH[:, q * HBLK:(q + 1) * HBLK, :]
                        g.wait_ge(sp, cp.n)
                        g.tensor_tensor(out=h2bf, in0=ts_s, in1=Cb, op=A.subtract
                                        ).then_inc(sp, 1)
                        cp.inc(1)
                        snap["h2", t, c] = cp.n

                    @block.sync
                    def _(sy, c=c, q=q, t=t):
                        sy.wait_ge(sp, snap["h2", t, c])
                        sy.dma_start(
                            out=h_hbm_v[:, c * HBLK:(c + 1) * HBLK, :],
                            in_=bf_view_at(sbA if q == 0 else sbB, 0, HBLK),
                        ).then_inc(shs[q], 16)
                        chp[q].inc(16)
                        snap["hdma", t, c] = chp[q].n

            snap["h_staged", t] = (chp[0].n, chp[1].n)

            # ---------------- V phase ----------------
            # gather hA -> AB; t1 = c2v*(1-g) -> Cb (Pool) || t2 = hA*g -> TS0
            # (DVE); c2v' = Cb+TS0 (DVE); u chain -> uslot; e1 = u_b - c2v'
            # -> TS0; t3 = TS0*g -> AB; t4 = v2c*(1-g) -> Cb (Pool);
            # v2c' = AB+Cb (DVE).
            def emit_v_gather(c, t):
                q = c % 2

                @block.gpsimd
                def _(g, c=c, q=q, t=t):
                    g.wait_ge(shs[0], snap["h_staged", t][0])
                    g.wait_ge(shs[1], snap["h_staged", t][1])
                    if ("war_vab_sv", t, c) in snap:
                        g.wait_ge(sv, snap["war_vab_sv", t, c])
                    dst = bf_view(sbA if q == 0 else sbB, CBLK)
                    if fake_gather:
                        g.dma_start(out=dst,
                                    in_=h_hbm.ap().rearrange("(blk p) b -> p blk b", p=128)[:, c * CBLK:(c + 1) * CBLK, :]
                                    ).then_inc(sg[0][q], 16)
                        cgq[0][q].inc(16)
                        snap["vgath_fake", t, c] = True
                        snap["vgath", t, c] = (cgq[0][q].n, cgq[1][q].n)
                    else:
                        g.dma_gather(
                            out_ap=dst,
                            in_ap=h_hbm[:, :],
                            idxs_ap=ir2a[:, c * (CBLK * 128 // 16):(c + 1) * (CBLK * 128 // 16)],
                            num_idxs=CBLK * 128,
                            num_idxs_reg=CBLK * 128,
                            elem_size=128,
                            single_packet=CBLK * 128 <= 1024,
                            queue_num=q,
                        ).then_inc(sg[q][q], 16)
                        cgq[q][q].inc(16)
                    snap["vgath", t, c] = (cgq[0][q].n, cgq[1][q].n)

            def emit_v_work(c, t):
                q = c % 2

                @block.gpsimd
                def _(g, c=c, q=q, t=t):
                    g.wait_ge(sv, cv.n)
                    if ("vdve", t, c - 1) in snap:
                        g.wait_ge(sv, snap["vdve", t, c - 1])
                    Cb = sbC[:, 0:CBLK, :]
                    c2v_c = c2v[:, c * CBLK:(c + 1) * CBLK, :]
                    g.wait_ge(sp, cp.n)
                    g.tensor_tensor(out=Cb, in0=c2v_c, in1=ogbc_b(CBLK), op=A.mult).then_inc(sp, 1)
                    cp.inc(1)
                    snap["vpool", t, c] = cp.n

                @block.vector
                def _(v, c=c, q=q, t=t):
                    v.wait_ge(sg[0][q], snap["vgath", t, c][0])
                    if ("vgath_fake", t, c) not in snap:
                        v.wait_ge(sg[1][q], snap["vgath", t, c][1])
                    AB = (sbA if q == 0 else sbB)[:, 0:CBLK, :]
                    Cb = sbC[:, 0:CBLK, :]
                    TS0 = tsH[:, 0:CBLK, :]
                    c2v_c = c2v[:, c * CBLK:(c + 1) * CBLK, :]
                    v2c_c = v2c[:, c * CBLK:(c + 1) * CBLK, :]
                    c2v_v = c2v.ap().rearrange("p (vh cc) b -> p vh cc b", cc=dc)[
                        :, c * VH_C:(c + 1) * VH_C, :, :]
                    v.wait_ge(sv, cv.n)
                    v.tensor_tensor(out=TS0, in0=bf_view(sbA if q == 0 else sbB, CBLK),
                                    in1=gbc_b(CBLK), op=A.mult).then_inc(sv, 1)
                    cv.inc(1)
                    v.wait_ge(sp, snap["vpool", t, c])
                    v.wait_ge(sv, cv.n)
                    v.tensor_tensor(out=c2v_c, in0=Cb, in1=TS0, op=A.add).then_inc(sv, 1)
                    cv.inc(1)
                    uslot = t6[:, (1 + (c % 2)) * VH_C:(2 + (c % 2)) * VH_C, :]
                    if ("odma", t, c - 2) in snap:
                        v.wait_ge(sos[q], snap["odma", t, c - 2])
                    v.wait_ge(sv, cv.n)
                    v.tensor_tensor(out=uslot, in0=chn_sb[:, c * VH_C:(c + 1) * VH_C, :],
                                    in1=c2v_v[:, :, 0, :], op=A.add).then_inc(sv, 1)
                    cv.inc(1)
                    v.wait_ge(sv, cv.n)
                    v.tensor_tensor(out=uslot, in0=uslot,
                                    in1=c2v_v[:, :, 1, :], op=A.add).then_inc(sv, 1)
                    cv.inc(1)
                    v.wait_ge(sv, cv.n)
                    v.tensor_tensor(out=uslot, in0=uslot,
                                    in1=c2v_v[:, :, 2, :], op=A.add).then_inc(sv, 1)
                    cv.inc(1)
                    snap["u", t, c] = cv.n
                    if t < T:
                        u_b = uslot.unsqueeze(2).broadcast_to([128, VH_C, dc, 128])
                        c2v_cv = c2v_c.rearrange("p (vh cc) b -> p vh cc b", cc=dc)
                        Tv0 = TS0.rearrange("p (vh cc) b -> p vh cc b", cc=dc)
                        v.wait_ge(sv, cv.n)
                        v.tensor_tensor(out=Tv0, in0=u_b, in1=c2v_cv, op=A.subtract).then_inc(sv, 1)
                        cv.inc(1)
                        v.wait_ge(sv, cv.n)
                        v.tensor_tensor(out=AB, in0=TS0, in1=gbc_b(CBLK), op=A.mult).then_inc(sv, 1)
                        cv.inc(1)
                        v.wait_ge(sv, cv.n)
                        v.tensor_tensor(out=Cb, in0=v2c_c, in1=ogbc_b(CBLK), op=A.mult).then_inc(sv, 1)
                        cv.inc(1)
                        v.wait_ge(sv, cv.n)
                        v.tensor_tensor(out=v2c_c, in0=AB, in1=Cb, op=A.add).then_inc(sv, 1)
                        cv.inc(1)
                    snap["vdve", t, c] = cv.n
                    snap["war_vab_sv", t, c + 2] = cv.n
                    if c == NCH_COL - 1:
                        snap["war_ab_sv", t + 1, 0] = cv.n
                        snap["war_ab_sv", t + 1, 1] = cv.n
                        snap["tsH_war_sv", t + 1, 0] = cv.n
                        snap["tsH_war_sv", t + 1, 1] = cv.n
                        snap["war_ab_sp", t + 1, 0] = cp.n
                        snap["war_ab_sp", t + 1, 1] = cp.n

                @block.sync
                def _(sy, c=c, q=q, t=t):
                    sy.wait_ge(sv, snap["u", t, c])
                    uslot = t6[:, (1 + (c % 2)) * VH_C:(2 + (c % 2)) * VH_C, :]
                    sy.dma_start(
                        out=out_v[(t - 1) % OSL, :, c * VH_C:(c + 1) * VH_C, :],
                        in_=uslot,
                    ).then_inc(sos[q], 16)
                    cop[q].inc(16)
                    snap["odma", t, c] = cop[q].n
                    if t < T:
                        sy.wait_ge(sv, snap["vdve", t, c])
                        sy.dma_start(
                            out=v2c_hbm_v[:, c * CBLK:(c + 1) * CBLK, :],
                            in_=v2c[:, c * CBLK:(c + 1) * CBLK, :],
                        ).then_inc(svs[q], 16)
                        cvs[q].inc(16)
                        snap["vstg", t, c] = cvs[q].n

            for c in range(NCH_COL):
                emit_v_gather(c, t)
                if c >= 1:
                    emit_v_work(c - 1, t)
            emit_v_work(NCH_COL - 1, t)

            snap["v2c_staged", t] = (cvs[0].n, cvs[1].n)

        if adapter_only:
            @block.sync
            def _(sy):
                sy.wait_ge(sgam, 32)
                sy.dma_start(out=out_ext[0:1, 0:128, 0:128],
                             in_=gbc.ap().unsqueeze(0)).then_inc(svs[0], 16)
                cvs[0].inc(16)

        # drain: ensure all DMAs retired before block end
        @block.sync
        def _(sy):
            sy.wait_ge(sos[0], cop[0].n)
            sy.wait_ge(sos[1], cop[1].n)
            sy.wait_ge(shs[0], chp[0].n)
            sy.wait_ge(shs[1], chp[1].n)
            sy.wait_ge(svs[0], cvs[0].n)
            sy.wait_ge(svs[1], cvs[1].n)

    return nc


# ----------------------------------------------------------------------------
# host side
# ----------------------------------------------------------------------------

def _enum_maps(col_idx, row_idx, N, M, dc, dr):
    E = col_idx.shape[0]
    # stable sort by node id; position within a node's group = occurrence order
    a_order = np.argsort(col_idx, kind="stable")     # a-pos -> edge
    r_order = np.argsort(row_idx, kind="stable")     # r-pos -> edge
    # jA for a-pos p = (v, c): v = p // dc, c = p % dc
    ap_ = np.arange(E)
    v = ap_ // dc
    ccol = ap_ % dc
    jA_of_apos = (v // 128) * (dc * 128) + ccol * 128 + (v % 128)
    m = ap_ // dr
    r = ap_ % dr
    jR_of_rpos = (m // 128) * (dr * 128) + r * 128 + (m % 128)
    jA_of_edge = np.empty(E, np.int64)
    jA_of_edge[a_order] = jA_of_apos
    jR_of_edge = np.empty(E, np.int64)
    jR_of_edge[r_order] = jR_of_rpos
    # gather idx arrays:  lamR[jR] = v2c_hbm[gA2R[jR]],  hA[jA] = h_hbm[gR2A[jA]]
    gA2R = np.empty(E, np.int64)
    gA2R[jR_of_edge] = jA_of_edge
    gR2A = np.empty(E, np.int64)
    gR2A[jA_of_edge] = jR_of_edge
    return gA2R.astype(np.int16), gR2A.astype(np.int16)


def _wrap16(idx):
    E = idx.shape[0]
    w = np.zeros((16, E // 16), np.int16)
    j = np.arange(E)
    w[j % 16, j // 16] = idx
    return np.tile(w, (8, 1))  # replicate for the 8 Q7 cores


_CACHE = {}
_LAST_IN_MAPS = None


def _get_graph(N, M, BS, T, dc, dr, NCH_ROW, NCH_COL, n_half):
    key = (N, M, BS, T, dc, dr, NCH_ROW, NCH_COL, n_half)
    if key not in _CACHE:
        nc = build_graph(N, M, BS, T, dc=dc, dr=dr, NCH_ROW=NCH_ROW,
                         NCH_COL=NCH_COL, n_half=n_half)
        nc.compile()
        _CACHE[key] = nc
    return _CACHE[key]


def kernel(**inputs):
    from concourse.bass_utils import run_bass_kernel_spmd

    chn_llr = np.asarray(inputs["chn_llr"], np.float32)
    W1 = np.asarray(inputs["W1"], np.float32)
    b1 = np.asarray(inputs["b1"], np.float32)
    W2 = np.asarray(inputs["W2"], np.float32)
    b2 = np.asarray(inputs["b2"], np.float32)
    col_idx = np.asarray(inputs["col_idx"])
    row_idx = np.asarray(inputs["row_idx"])
    N = int(inputs["n_var"])
    M = int(inputs["n_chk"])
    T = int(inputs["T"])
    B = chn_llr.shape[1]
    n_cores = 8
    BS = B // n_cores
    dc = col_idx.shape[0] // N
    dr = col_idx.shape[0] // M

    gA2R, gR2A = _enum_maps(np.asarray(col_idx), np.asarray(row_idx), N, M, dc, dr)
    ia2r = _wrap16(gA2R)
    ir2a = _wrap16(gR2A)
    import ml_dtypes
    w1t = np.ascontiguousarray(W1.T.astype(ml_dtypes.bfloat16))

    nc = _get_graph(N, M, 128, T, dc, dr, 2, 4, 2)

    in_maps = []
    for c in range(n_cores):
        shard = chn_llr[:, c * BS:(c + 1) * BS]           # [N, BS]
        chn_pm = np.ascontiguousarray(
            shard.reshape(N // 128, 128, BS).transpose(1, 0, 2).reshape(128, -1))
        in_maps.append({
            "chn": chn_pm,
            "w1t": w1t,
            "b1": b1.reshape(1, N),
            "w2": W2,
            "b2": b2.reshape(1, 3),
            "idx_a2r": ia2r,
            "idx_r2a": ir2a,
        })

    global _LAST_IN_MAPS
    _LAST_IN_MAPS = in_maps
    res = run_bass_kernel_spmd(nc, in_maps, core_ids=list(range(n_cores)))
    outs = [r["out"].reshape(T, N, BS) for r in res.results]
    return np.concatenate(outs, axis=2).astype(np.float32)



# revision 16
# speedup vs baseline: 1.6858x; 1.6858x over previous
"""AdaBP decoder (gnn_message_passing) Trainium2 Bass kernel, 8 NeuronCores.

Sharding: data-parallel over batch B=1024 across 8 cores (128 samples/core,
matching the 128 SBUF partitions); adapter weights and edge-index tables are
replicated (per the problem's sharding hint). No collectives needed.

On-core layout "Q": edge tensors are partition-minor over a chosen edge
enumeration j -> (partition j%128, free block j//128) with the 128 batch
samples innermost:
  col order jA: edge (v,c) -> (v//128)*dc*128 + c*128 + v%128
  row order jR: edge (m,r) -> (m//128)*dr*128 + r*128 + m%128
The col<->row edge permutation each BP iteration stages the edge tensor to
HBM as bf16 [E,128] rows and pulls it back with gpsimd.dma_gather (256B row
per edge). Gather index tables are precomputed on the host (int16,
16-partition-wrapped, replicated across the 8 GPSIMD Q7 cores).

v2 changes vs the first working version (~2x fewer DVE-cycles/iter and no
act-table loads on the critical path):
  - ACT table sets: tanh and Ln live in different sets; the H phase now does
    all 4 chunk tanhs first, then all 8 Lns, so only 2 LoadActFuncSet per
    iteration, both hidden behind DVE/V-phase work (was 4, serialized).
  - num/den (1 +- (1-eps)q) folded into the Ln activations' free scale/bias.
  - lam clip done in bf16 (4x TS mode) on the gather output before tanh
    (tanh(clip(lam)/2) == clip(tanh(lam/2)) by monotonicity).
  - product tree: 12 small mults -> 4 ops (paired via strided/negative-stride
    APs; the leave-one-out mults are ONE 24-block op).
  - V phase damping mixes all in bf16 (DVE 2x mode): c2v/v2c/gamma/u all
    bf16; messages already round-trip through bf16 in HBM, so this adds
    little noise. The tanh-product pipeline (t, tree, q', Ln) stays f32:
    atanh amplifies errors ~1/(1-q^2) near saturation, so bf16 there would
    be catastrophic.
  - outputs u = chn + colsum(c2v) stored/DMA'd as bf16, cast to f32 on host.
  - h2 = L1 - L2 runs on Pool into a dedicated bf16 staging buffer (keeps
    the gather buffers free; breaks the deadlock the ACT reorder would
    otherwise cause with 2-parity buffering).
Engine budget per iteration (cost model): DVE ~95us, Pool ~50us (8 gathers +
4 h2), ACT ~35us, all overlapped across a 2-parity software pipeline.
"""
import math
import sys

sys.path.insert(0, "/opt/trn_rl_repo")

import numpy as np
from contextlib import ExitStack

from concourse import bass, bacc, mybir

F32 = mybir.dt.float32
BF16 = mybir.dt.bfloat16
I16 = mybir.dt.int16

LLR_CLIP = 15.0
LOG_TANH_LO = -float(np.log(np.tanh(LLR_CLIP / 2)))
EPS = 1e-6
A = mybir.AluOpType
AF = mybir.ActivationFunctionType


class Ctr:
    def __init__(self):
        self.n = 0

    def inc(self, k=1):
        self.n += k
        return self.n


def build_graph(N, M, BS, T, dc=3, dr=6, NCH_ROW=2, NCH_COL=4, n_half=2,
                skip_adapter=False, adapter_only=False, out_slots=None,
                fake_gather=False, loo_pool=False):
    """Build the Bacc graph for one core (SPMD: same graph on all 8 cores).

    NCH_ROW/n_half are kept for signature compat: NHC = NCH_ROW * n_half is
    the number of H chunks (all tanh'd before any Ln to bound table loads).
    """
    E = N * dc
    assert M * dr == E and N % 128 == 0 and M % 128 == 0 and BS == 128
    VH, MH, NBLK = N // 128, M // 128, E // 128
    NHC = NCH_ROW * n_half          # H chunks
    assert MH % NHC == 0
    MH_C = MH // NHC                # m-groups per H chunk
    HBLK = MH_C * dr                # edge blocks per H chunk
    assert VH % NCH_COL == 0
    VH_C = VH // NCH_COL            # v-groups per V chunk
    CBLK = VH_C * dc                # edge blocks per V chunk
    assert HBLK == CBLK, "shared scratch sizing assumes equal chunk sizes"
    HB2 = HBLK // 2                 # f32 blocks backing one bf16 chunk

    NT = max(2, N // 512)
    PW = N // NT
    assert PW <= 512
    KT = N // 128
    NT2 = NT // 2
    NWH = N // 2

    nc = bacc.Bacc(num_swdge_queues=2)

    chn_ext = nc.declare_dram_parameter("chn", [128, VH * 128], F32, isOutput=False)
    w1t_ext = nc.declare_dram_parameter("w1t", [N, N], BF16, isOutput=False)
    b1_ext = nc.declare_dram_parameter("b1", [1, N], F32, isOutput=False)
    w2_ext = nc.declare_dram_parameter("w2", [3, N], F32, isOutput=False)
    b2_ext = nc.declare_dram_parameter("b2", [1, 3], F32, isOutput=False)
    ia2r_ext = nc.declare_dram_parameter("idx_a2r", [128, E // 16], I16, isOutput=False)
    ir2a_ext = nc.declare_dram_parameter("idx_r2a", [128, E // 16], I16, isOutput=False)
    OSL = out_slots or T
    out_ext = nc.declare_dram_parameter("out", [OSL, N, 128], BF16, isOutput=True)

    v2c_hbm = nc.dram_tensor("v2c_hbm", [E, 128], BF16)  # col (jA) order rows
    h_hbm = nc.dram_tensor("h_hbm", [E, 128], BF16)      # row (jR) order rows
    gam_dram = nc.dram_tensor("gam_dram", [128, 1], F32)

    out_v = out_ext.ap().rearrange("t (vh p) b -> t p vh b", p=128)
    v2c_hbm_v = v2c_hbm.ap().rearrange("(blk p) b -> p blk b", p=128)
    h_hbm_v = h_hbm.ap().rearrange("(blk p) b -> p blk b", p=128)

    es = ExitStack()
    with es:
        block = es.enter_context(nc.Block())
        sl = es.enter_context(nc.semaphore("sl"))      # input loads
        sw = [es.enter_context(nc.semaphore(f"sw{i}")) for i in range(2)]
        sgam = es.enter_context(nc.semaphore("sgam"))  # gamma roundtrip
        svs = [es.enter_context(nc.semaphore(f"svs{i}")) for i in range(2)]
        shs = [es.enter_context(nc.semaphore(f"shs{i}")) for i in range(2)]
        sos = [es.enter_context(nc.semaphore(f"sos{i}")) for i in range(2)]
        sg = [[es.enter_context(nc.semaphore(f"sg{i}_{j}")) for j in range(2)] for i in range(2)]
        sv = es.enter_context(nc.semaphore("sv"))      # DVE (+1)
        sa = es.enter_context(nc.semaphore("sa"))      # ACT (+1)
        sm = es.enter_context(nc.semaphore("sm"))      # PE (+1)
        sp = es.enter_context(nc.semaphore("sp"))      # Pool compute (+1)

        chn_sb = es.enter_context(nc.sbuf_tensor("chn_sb", [128, VH, 128], F32))
        chnb = es.enter_context(nc.sbuf_tensor("chnb", [128, VH, 128], BF16))
        v2c = es.enter_context(nc.sbuf_tensor("v2c", [128, NBLK, 128], BF16))
        c2v = es.enter_context(nc.sbuf_tensor("c2v", [128, NBLK, 128], BF16))
        tsH = es.enter_context(nc.sbuf_tensor("tsH", [128, NHC * HBLK, 128], F32))
        sbA = es.enter_context(nc.sbuf_tensor("sbA", [128, HB2, 128], F32))
        sbB = es.enter_context(nc.sbuf_tensor("sbB", [128, HB2, 128], F32))
        sbC = es.enter_context(nc.sbuf_tensor("sbC", [128, 2 * HBLK, 128], F32))
        hstg = es.enter_context(nc.sbuf_tensor("hstg", [128, 2 * HBLK, 128], BF16))
        t6 = es.enter_context(nc.sbuf_tensor("t6", [128, MH_C * dr, 128], F32))
        gbc = es.enter_context(nc.sbuf_tensor("gbc", [128, 128], F32))
        ogbc = es.enter_context(nc.sbuf_tensor("ogbc", [128, 128], F32))
        gbch = es.enter_context(nc.sbuf_tensor("gbch", [128, 128], BF16))
        ogbch = es.enter_context(nc.sbuf_tensor("ogbch", [128, 128], BF16))
        ia2r = es.enter_context(nc.sbuf_tensor("ia2r", [128, E // 16], I16))
        ir2a = es.enter_context(nc.sbuf_tensor("ir2a", [128, E // 16], I16))
        zacc = es.enter_context(nc.sbuf_tensor("zacc", [128, 3, NT], F32))
        zred = es.enter_context(nc.sbuf_tensor("zred", [128, 4], F32))
        b2bc = es.enter_context(nc.sbuf_tensor("b2bc", [128, 3], F32))
        ps = [es.enter_context(nc.psum_tensor(f"ps{i}", [128, PW], F32)) for i in range(NT)]

        cv, ca, cm, cp = Ctr(), Ctr(), Ctr(), Ctr()
        cw = [Ctr(), Ctr()]
        cvs = [Ctr(), Ctr()]
        cgq = [[Ctr(), Ctr()], [Ctr(), Ctr()]]  # [queue][parity]
        chp = [Ctr(), Ctr()]
        cop = [Ctr(), Ctr()]

        # ---------------- SBUF aliases ----------------
        v2c_flat = v2c.ap().rearrange("p a b -> p (a b)")        # bf16 elems
        v2c_f32 = v2c_flat.bitcast(F32)
        assert N <= NBLK * 64
        b1bc = v2c_f32[:, 0:N]
        assert 3 * N <= NBLK * 128
        chn_bf = v2c_flat[:, 2 * N:3 * N]
        tsH_flat = tsH.ap().rearrange("p a b -> p (a b)")        # f32 elems
        assert 3 * N <= NHC * HBLK * 128
        h_adapt = tsH_flat[:, 0:N]
        sbC_flat = sbC.ap().rearrange("p a b -> p (a b)")
        t6_flat = t6.ap().rearrange("p a b -> p (a b)")

        def w2bc_bank(j, bank):
            if j < 2:
                return tsH_flat[:, (j + 1) * N + bank * PW:(j + 1) * N + (bank + 1) * PW]
            if bank < 6:
                return sbC_flat[:, bank * PW:(bank + 1) * PW]
            return t6_flat[:, (bank - 6) * PW:(bank - 5) * PW]

        sbA_bf = sbA.ap().rearrange("p a b -> p (a b)").bitcast(BF16)
        sbB_bf = sbB.ap().rearrange("p a b -> p (a b)").bitcast(BF16)
        assert NWH <= 128 * HB2 * 2
        wbuf = [sbA_bf[:, 0:NWH], sbB_bf[:, 0:NWH]]
        hstg_flat = hstg.ap().rearrange("p a b -> p (a b)")
        dummy = hstg_flat.bitcast(F32)[:, 0:PW]

        def AB_bf(q, nblk=None):
            flat = sbA_bf if q == 0 else sbB_bf
            nb = nblk or HBLK
            return flat[:, 0:nb * 128].rearrange("p (a b) -> p a b", b=128)

        ts_slot = lambda c: tsH[:, c * HBLK:(c + 1) * HBLK, :]
        tsH_bf = tsH_flat.bitcast(BF16)
        TS0b = lambda q: tsH_bf[:, q * HBLK * 128:(q + 1) * HBLK * 128].rearrange(
            "p (a b) -> p a b", b=128)
        sbC_bf = sbC_flat.bitcast(BF16)
        Cbb = lambda q: sbC_bf[:, q * HBLK * 128:(q + 1) * HBLK * 128].rearrange(
            "p (a b) -> p a b", b=128)
        t6_bf = t6_flat.bitcast(BF16)
        uslot_of = lambda q: t6_bf[:, (2 + q) * VH_C * 128:(3 + q) * VH_C * 128].rearrange(
            "p (a b) -> p a b", b=128)
        hstg_slot = lambda q: hstg[:, q * HBLK:(q + 1) * HBLK, :]

        gbch_b = lambda nb: gbch.ap().unsqueeze(1).broadcast_to([128, nb, 128])
        ogbch_b = lambda nb: ogbch.ap().unsqueeze(1).broadcast_to([128, nb, 128])

        t6v = t6.ap().rearrange("p (mh r) b -> p mh r b", r=dr)

        snap = {}

        # ============ sync: input loads + adapter W stream ============
        @block.sync
        def _(sy):
            sy.dma_start(out=chn_sb[:, :, :],
                         in_=chn_ext.ap().rearrange("p (a b) -> p a b", b=128)
                         ).then_inc(sl, 16)
            sy.dma_start(out=ia2r[:, :], in_=ia2r_ext[:, :]).then_inc(sl, 16)
            sy.dma_start(out=ir2a[:, :], in_=ir2a_ext[:, :]).then_inc(sl, 16)
            if skip_adapter:
                return
            sy.dma_start(out=b1bc,
                         in_=bass.AP(b1_ext, 0, [[0, 128], [1, N]])).then_inc(sl, 16)
            for j in range(2):
                sy.dma_start(out=tsH_flat[:, (j + 1) * N:(j + 2) * N],
                             in_=bass.AP(w2_ext, j * N, [[0, 128], [1, N]])
                             ).then_inc(sl, 16)
            sy.dma_start(out=sbC_flat[:, 0:6 * PW],
                         in_=bass.AP(w2_ext, 2 * N, [[0, 128], [1, 6 * PW]])
                         ).then_inc(sl, 16)
            sy.dma_start(out=t6_flat[:, 0:2 * PW],
                         in_=bass.AP(w2_ext, 2 * N + 6 * PW, [[0, 128], [1, 2 * PW]])
                         ).then_inc(sl, 16)
            sy.dma_start(out=b2bc[:, :],
                         in_=bass.AP(b2_ext, 0, [[0, 128], [1, 3]])).then_inc(sl, 16)
            for k in range(KT):
                for hf in range(2):
                    i = k * 2 + hf
                    if i % 2 != 0:
                        continue  # odd half-tiles stream from the ACT HWDGE queue
                    if i >= 2:
                        sy.wait_ge(sm, (i - 1) * NT2)
                    sy.dma_start(
                        out=wbuf[i % 2],
                        in_=w1t_ext[k * 128:(k + 1) * 128, hf * NWH:(hf + 1) * NWH],
                    ).then_inc(sw[i % 2], 16)
                    cw[i % 2].inc(16)

        # ============ ACT: cast chn to bf16 (matmul + u-chain copies) ====
        @block.scalar
        def _(s):
            s.wait_ge(sl, 48 if skip_adapter else 144)
            s.wait_ge(sa, ca.n)
            s.activation(out=chnb[:, :, :], in_=chn_sb[:, :, :],
                         func=AF.Copy).then_inc(sa, 1)
            ca.inc(1)
            if skip_adapter:
                return
            s.wait_ge(sa, ca.n)
            s.activation(out=chn_bf.rearrange("p (a b) -> p a b", b=128),
                         in_=chn_sb[:, :, :], func=AF.Copy).then_inc(sa, 1)
            ca.inc(1)
            snap["chn_bf"] = ca.n

        # ============ ACT HWDGE: odd W1 half-tiles (parallel queue) ========
        @block.scalar
        def _(s):
            if skip_adapter:
                return
            for k in range(KT):
                for hf in range(2):
                    i = k * 2 + hf
                    if i % 2 != 1:
                        continue
                    if i >= 2:
                        s.wait_ge(sm, (i - 1) * NT2)
                    s.dma_start(
                        out=wbuf[i % 2],
                        in_=w1t_ext[k * 128:(k + 1) * 128, hf * NWH:(hf + 1) * NWH],
                    ).then_inc(sw[i % 2], 16)
                    cw[i % 2].inc(16)

        # ============ PE: adapter matmuls ============
        @block.tensor
        def _(pe):
            if skip_adapter:
                snap["mm_all"] = 0
                return
            pe.wait_ge(sl, 144)
            pe.wait_ge(sa, snap["chn_bf"])
            for k in range(KT):
                for hf in range(2):
                    i = k * 2 + hf
                    pe.wait_ge(sw[i % 2], 16 * (i // 2 + 1))
                    for n0 in range(NT2):
                        bank = hf * NT2 + n0
                        pe.matmul(
                            ps[bank][:, :],
                            chn_bf.rearrange("p (a b) -> p a b", b=128)[:, k, :],
                            wbuf[i % 2].rearrange("p (a b) -> p a b", a=NT2)[:, n0, :],
                            start=(k == 0),
                            stop=(k == KT - 1),
                        ).then_inc(sm, 1)
                        cm.inc(1)
            snap["mm_all"] = cm.n

        # ============ DVE: adapter epilogue ============
        @block.vector
        def _(v):
            if skip_adapter:
                return
            v.wait_ge(sm, snap["mm_all"])
            for bank in range(NT):
                v.wait_ge(sv, cv.n)
                v.tensor_tensor(
                    out=h_adapt[:, bank * PW:(bank + 1) * PW],
                    in0=ps[bank][:, :],
                    in1=b1bc[:, bank * PW:(bank + 1) * PW],
                    op=A.add,
                ).then_inc(sv, 1)
                cv.inc(1)
            for j in range(3):
                for bank in range(NT):
                    v.wait_ge(sv, cv.n)
                    v.scalar_tensor_tensor(
                        out=dummy,
                        in0=h_adapt[:, bank * PW:(bank + 1) * PW],
                        scalar=0.0,
                        in1=w2bc_bank(j, bank),
                        op0=A.max,
                        op1=A.mult,
                        accum_out=zacc[:, j, bank:bank + 1],
                    ).then_inc(sv, 1)
                    cv.inc(1)
            for j in range(3):
                v.wait_ge(sv, cv.n)
                v.tensor_reduce(
                    out=zred[:, j:j + 1], in_=zacc[:, j, :],
                    axis=mybir.AxisListType.X, op=A.add,
                ).then_inc(sv, 1)
                cv.inc(1)
            snap["zred"] = cv.n

        # ============ ACT: gamma ============
        @block.scalar
        def _(s):
            if skip_adapter:
                return
            s.wait_ge(sv, snap["zred"])
            s.wait_ge(sa, ca.n)
            s.activation(out=zred[:, 3:4], in_=zred[:, 0:1],
                         func=AF.Sigmoid, bias=b2bc[:, 0:1]).then_inc(sa, 1)
            ca.inc(1)
            snap["gam_col"] = ca.n

        # ============ sync: gamma roundtrip ============
        @block.sync
        def _(sy):
            if skip_adapter:
                return
            sy.wait_ge(sa, snap["gam_col"])
            sy.dma_start(out=gam_dram[:, :], in_=zred[:, 3:4]).then_inc(sgam, 16)
            sy.wait_ge(sgam, 16)
            sy.dma_start(out=gbc[:, :],
                         in_=bass.AP(gam_dram, 0, [[0, 128], [1, 128]])
                         ).then_inc(sgam, 16)

        # ============ DVE: prologue ============
        @block.vector
        def _(v):
            if skip_adapter:
                v.wait_ge(sl, 48)
                v.memset(gbc[:, :], 0.5).then_inc(sv, 1)
                cv.inc(1)
            else:
                v.wait_ge(sgam, 32)
            v.wait_ge(sv, cv.n)
            v.tensor_scalar(out=ogbc[:, :], in0=gbc[:, :], scalar1=-1.0,
                            scalar2=1.0, op0=A.mult, op1=A.add).then_inc(sv, 1)
            cv.inc(1)
            v.wait_ge(sv, cv.n)
            v.tensor_scalar(out=gbch[:, :], in0=gbc[:, :], scalar1=1.0,
                            scalar2=None, op0=A.mult).then_inc(sv, 1)
            cv.inc(1)
            v.wait_ge(sv, cv.n)
            v.tensor_scalar(out=ogbch[:, :], in0=ogbc[:, :], scalar1=1.0,
                            scalar2=None, op0=A.mult).then_inc(sv, 1)
            cv.inc(1)
            v.wait_ge(sv, cv.n)
            v.memset(c2v[:, :, :], 0.0).then_inc(sv, 1)
            cv.inc(1)
            v2c_v = v2c.ap().rearrange("p (vh c) b -> p vh c b", c=dc)
            chn_b = chn_sb.ap().unsqueeze(2).broadcast_to([128, VH, dc, 128])
            g_b = gbc.ap().unsqueeze(1).unsqueeze(1).broadcast_to([128, VH, dc, 128])
            v.wait_ge(sv, cv.n)
            v.tensor_tensor(out=v2c_v, in0=chn_b, in1=g_b, op=A.mult).then_inc(sv, 1)
            cv.inc(1)
            snap["prologue_v"] = cv.n

        # ============ sync: stage v2c(1) ============
        @block.sync
        def _(sy):
            sy.wait_ge(sv, snap["prologue_v"])
            sy.dma_start(out=v2c_hbm_v, in_=v2c[:, :, :]).then_inc(svs[0], 16)
            cvs[0].inc(16)
            snap["v2c_staged", 0] = (cvs[0].n, cvs[1].n)

        # ======================= MAIN LOOP =======================
        for t in range(1, (0 if adapter_only else T) + 1):
            # ---------------- H phase ----------------
            # Loop A: gathers + clips + tanhs (ACT: all tanh before any Ln)
            for c in range(NHC):
                q = c % 2

                @block.gpsimd
                def _(g, c=c, q=q, t=t):
                    g.wait_ge(svs[0], snap["v2c_staged", t - 1][0])
                    g.wait_ge(svs[1], snap["v2c_staged", t - 1][1])
                    if t == 1 and c < 2:
                        g.wait_ge(sm, snap["mm_all"])
                    if c >= 2:
                        g.wait_ge(sa, snap["tanh", t, c - 2])  # AB WAR
                    elif ("war_ab_sv", t, q) in snap:
                        g.wait_ge(sv, snap["war_ab_sv", t, q])
                    dst = AB_bf(q)
                    if fake_gather:
                        g.dma_start(out=dst,
                                    in_=v2c_hbm_v[:, c * HBLK:(c + 1) * HBLK, :]
                                    ).then_inc(sg[0][q], 16)
                        cgq[0][q].inc(16)
                        snap["gath_fake", t, c] = True
                    else:
                        g.dma_gather(
                            out_ap=dst,
                            in_ap=v2c_hbm[:, :],
                            idxs_ap=ia2r[:, c * (HBLK * 128 // 16):(c + 1) * (HBLK * 128 // 16)],
                            num_idxs=HBLK * 128,
                            num_idxs_reg=HBLK * 128,
                            elem_size=128,
                            single_packet=False,
                            queue_num=q,
                        ).then_inc(sg[q][q], 16)
                        cgq[q][q].inc(16)
                    snap["gath", t, c] = (cgq[0][q].n, cgq[1][q].n)

                @block.vector
                def _(v, c=c, q=q, t=t):
                    v.wait_ge(sg[0][q], snap["gath", t, c][0])
                    if ("gath_fake", t, c) not in snap:
                        v.wait_ge(sg[1][q], snap["gath", t, c][1])
                    v.wait_ge(sv, cv.n)
                    ab = AB_bf(q)
                    v.tensor_scalar(out=ab, in0=ab, scalar1=LLR_CLIP,
                                    scalar2=-LLR_CLIP, op0=A.min, op1=A.max
                                    ).then_inc(sv, 1)
                    cv.inc(1)
                    snap["clip", t, c] = cv.n

                @block.scalar
                def _(s, c=c, q=q, t=t):
                    s.wait_ge(sv, snap["clip", t, c])
                    if ("h2", t - 1, c) in snap:
                        sem, val = snap["h2", t - 1, c]       # tsH[c] WAR (h2 read)
                        s.wait_ge(sp if sem == "sp" else sv, val)
                    if c == 0 and ("tsv_war", t, 0) in snap:
                        s.wait_ge(sv, snap["tsv_war", t, 0])  # tsH[0] WAR (V bf16 use)
                    s.wait_ge(sa, ca.n)
                    s.activation(out=ts_slot(c), in_=AB_bf(q), func=AF.Tanh,
                                 scale=0.5).then_inc(sa, 1)
                    ca.inc(1)
                    snap["tanh", t, c] = ca.n

            # Loop B: tree/loo (DVE) + Ln pairs (ACT) + h2 (Pool; last chunk
            # on DVE to shorten the tail that gates the V gathers) + h dma
            for c in range(NHC):
                q = c % 2
                qp = sbC[:, q * HBLK:(q + 1) * HBLK, :]   # q' parity slot

                @block.vector
                def _(v, c=c, q=q, qp=qp, t=t):
                    v.wait_ge(sa, snap["tanh", t, c])
                    if c == 0:
                        if ("odma", t - 1, NCH_COL - 2) in snap:
                            v.wait_ge(sos[NCH_COL % 2], snap["odma", t - 1, NCH_COL - 2])
                        if ("odma", t - 1, NCH_COL - 1) in snap:
                            v.wait_ge(sos[(NCH_COL - 1) % 2], snap["odma", t - 1, NCH_COL - 1])
                    tsv = ts_slot(c).rearrange("p (mh pr two) b -> p mh pr two b", pr=3, two=2)
                    v.wait_ge(sv, cv.n)
                    v.tensor_tensor(out=t6v[:, :, 0:3, :],
                                    in0=tsv[:, :, :, 0, :], in1=tsv[:, :, :, 1, :],
                                    op=A.mult).then_inc(sv, 1)
                    cv.inc(1)
                    v.wait_ge(sv, cv.n)
                    v.tensor_tensor(out=t6v[:, :, 3:5, :],
                                    in0=t6v[:, :, 0:1, :].broadcast_to([128, MH_C, 2, 128]),
                                    in1=t6v[:, :, 1:3, :], op=A.mult).then_inc(sv, 1)
                    cv.inc(1)
                    v.wait_ge(sv, cv.n)
                    v.tensor_tensor(out=t6v[:, :, 5:6, :],
                                    in0=t6v[:, :, 1:2, :], in1=t6v[:, :, 2:3, :],
                                    op=A.mult).then_inc(sv, 1)
                    cv.inc(1)
                    # sbC(q) WAR: chunk c-2 consumers (Ln in-place + h2 read)
                    if ("lns", t, c - 2) in snap:
                        v.wait_ge(sa, snap["lns", t, c - 2])
                    hprev = ("h2", t, c - 2) if c >= 2 else ("h2", t - 1, c + 2)
                    if hprev in snap and snap[hprev][0] == "sp":
                        v.wait_ge(sp, snap[hprev][1])
                    # leave-one-out products, split even/odd r so every AP
                    # stays <=3 free dims (walrus ISA limit)
                    qpv = qp.rearrange("p (mh r) b -> p mh r b", r=dr)
                    tsr = ts_slot(c).rearrange("p (mh r) b -> p mh r b", r=dr)
                    t6rep = t6v[:, :, 5:2:-1, :]
                    v.wait_ge(sv, cv.n)
                    v.tensor_tensor(out=qpv[:, :, 0::2, :], in0=tsr[:, :, 1::2, :],
                                    in1=t6rep, op=A.mult).then_inc(sv, 1)
                    cv.inc(1)
                    v.wait_ge(sv, cv.n)
                    v.tensor_tensor(out=qpv[:, :, 1::2, :], in0=tsr[:, :, 0::2, :],
                                    in1=t6rep, op=A.mult).then_inc(sv, 1)
                    cv.inc(1)
                    snap["loo", t, c] = cv.n

                @block.scalar
                def _(s, c=c, q=q, qp=qp, t=t):
                    s.wait_ge(sv, snap["loo", t, c])
                    s.wait_ge(sa, ca.n)
                    s.activation(out=ts_slot(c), in_=qp, func=AF.Ln,
                                 scale=(1.0 - EPS), bias=1.0).then_inc(sa, 1)
                    ca.inc(1)
                    s.wait_ge(sa, ca.n)
                    s.activation(out=qp, in_=qp, func=AF.Ln,
                                 scale=-(1.0 - EPS), bias=1.0).then_inc(sa, 1)
                    ca.inc(1)
                    snap["lns", t, c] = ca.n

                if c < NHC - 1:
                    @block.gpsimd
                    def _(g, c=c, q=q, qp=qp, t=t):
                        g.wait_ge(sa, snap["lns", t, c])
                        prev = ("hdma", t, c - 2) if c >= 2 else ("hdma", t - 1, c + 2)
                        if prev in snap:
                            g.wait_ge(shs[q], snap[prev])         # hstg WAR
                        g.wait_ge(sp, cp.n)
                        g.tensor_tensor(out=hstg_slot(q), in0=ts_slot(c),
                                        in1=qp, op=A.subtract).then_inc(sp, 1)
                        cp.inc(1)
                        snap["h2", t, c] = ("sp", cp.n)
                else:
                    # h2 for the last chunk on DVE (shorter V-gate tail)
                    @block.vector
                    def _(v, c=c, q=q, qp=qp, t=t):
                        v.wait_ge(sa, snap["lns", t, c])
                        prev = ("hdma", t, c - 2)
                        if prev in snap:
                            v.wait_ge(shs[q], snap[prev])
                        v.wait_ge(sv, cv.n)
                        v.tensor_tensor(out=hstg_slot(q), in0=ts_slot(c),
                                        in1=qp, op=A.subtract).then_inc(sv, 1)
                        cv.inc(1)
                        snap["h2", t, c] = ("sv", cv.n)

                @block.sync
                def _(sy, c=c, q=q, t=t):
                    sem, val = snap["h2", t, c]
                    sy.wait_ge(sp if sem == "sp" else sv, val)
                    sy.dma_start(
                        out=h_hbm_v[:, c * HBLK:(c + 1) * HBLK, :],
                        in_=hstg_slot(q),
                    ).then_inc(shs[q], 16)
                    chp[q].inc(16)
                    snap["hdma", t, c] = chp[q].n

            snap["h_staged", t] = (chp[0].n, chp[1].n)

            # ---------------- V phase ----------------
            def emit_v_gather(c, t):
                q = c % 2

                @block.gpsimd
                def _(g, c=c, q=q, t=t):
                    g.wait_ge(shs[0], snap["h_staged", t][0])
                    g.wait_ge(shs[1], snap["h_staged", t][1])
                    if ("vdve", t, c - 2) in snap:
                        g.wait_ge(sv, snap["vdve", t, c - 2])  # AB WAR
                    dst = AB_bf(q, CBLK)
                    if fake_gather:
                        g.dma_start(out=dst,
                                    in_=h_hbm_v[:, c * CBLK:(c + 1) * CBLK, :]
                                    ).then_inc(sg[0][q], 16)
                        cgq[0][q].inc(16)
                        snap["vgath_fake", t, c] = True
                    else:
                        g.dma_gather(
                            out_ap=dst,
                            in_ap=h_hbm[:, :],
                            idxs_ap=ir2a[:, c * (CBLK * 128 // 16):(c + 1) * (CBLK * 128 // 16)],
                            num_idxs=CBLK * 128,
                            num_idxs_reg=CBLK * 128,
                            elem_size=128,
                            single_packet=False,
                            queue_num=q,
                        ).then_inc(sg[q][q], 16)
                        cgq[q][q].inc(16)
                    snap["vgath", t, c] = (cgq[0][q].n, cgq[1][q].n)

            def emit_v_work(c, t):
                q = c % 2

                @block.vector
                def _(v, c=c, q=q, t=t):
                    v.wait_ge(sg[0][q], snap["vgath", t, c][0])
                    if ("vgath_fake", t, c) not in snap:
                        v.wait_ge(sg[1][q], snap["vgath", t, c][1])
                    # tsH[0]/sbC bf16 scratch WAR vs H-phase Pool h2 reads
                    v.wait_ge(sp, snap["h2", t, NHC - 2][1])
                    AB = AB_bf(q, CBLK)
                    c2v_c = c2v[:, c * CBLK:(c + 1) * CBLK, :]
                    v2c_c = v2c[:, c * CBLK:(c + 1) * CBLK, :]
                    c2v_v = c2v.ap().rearrange("p (vh cc) b -> p vh cc b", cc=dc)[
                        :, c * VH_C:(c + 1) * VH_C, :, :]
                    us = uslot_of(q)
                    v.wait_ge(sv, cv.n)
                    v.tensor_tensor(out=TS0b(q), in0=AB, in1=gbch_b(CBLK),
                                    op=A.mult).then_inc(sv, 1)
                    cv.inc(1)
                    v.wait_ge(sv, cv.n)
                    v.tensor_tensor(out=Cbb(q), in0=c2v_c, in1=ogbch_b(CBLK),
                                    op=A.mult).then_inc(sv, 1)
                    cv.inc(1)
                    v.wait_ge(sv, cv.n)
                    v.tensor_tensor(out=c2v_c, in0=Cbb(q), in1=TS0b(q),
                                    op=A.add).then_inc(sv, 1)
                    cv.inc(1)
                    if ("odma", t, c - 2) in snap:
                        v.wait_ge(sos[q], snap["odma", t, c - 2])
                    v.wait_ge(sv, cv.n)
                    v.tensor_tensor(out=us, in0=chnb[:, c * VH_C:(c + 1) * VH_C, :],
                                    in1=c2v_v[:, :, 0, :], op=A.add).then_inc(sv, 1)
                    cv.inc(1)
                    v.wait_ge(sv, cv.n)
                    v.tensor_tensor(out=us, in0=us, in1=c2v_v[:, :, 1, :],
                                    op=A.add).then_inc(sv, 1)
                    cv.inc(1)
                    v.wait_ge(sv, cv.n)
                    v.tensor_tensor(out=us, in0=us, in1=c2v_v[:, :, 2, :],
                                    op=A.add).then_inc(sv, 1)
                    cv.inc(1)
                    snap["u", t, c] = cv.n
                    snap["c2vp", t, c] = cv.n

                if t < T:
                    # t4 = (1-gamma)*v2c on Pool, concurrent with the u-chain
                    @block.gpsimd
                    def _(g, c=c, q=q, t=t):
                        v2c_c = v2c[:, c * CBLK:(c + 1) * CBLK, :]
                        g.wait_ge(sv, snap["c2vp", t, c])  # Cbb WAR (c2v' read it)
                        g.wait_ge(sp, cp.n)
                        g.tensor_tensor(out=Cbb(q), in0=v2c_c, in1=ogbch_b(CBLK),
                                        op=A.mult).then_inc(sp, 1)
                        cp.inc(1)
                        snap["t4", t, c] = cp.n

                @block.vector
                def _(v, c=c, q=q, t=t):
                    AB = AB_bf(q, CBLK)
                    c2v_c = c2v[:, c * CBLK:(c + 1) * CBLK, :]
                    v2c_c = v2c[:, c * CBLK:(c + 1) * CBLK, :]
                    us = uslot_of(q)
                    if t < T:
                        u_b = us.unsqueeze(2).broadcast_to([128, VH_C, dc, 128])
                        c2v_cv = c2v_c.rearrange("p (vh cc) b -> p vh cc b", cc=dc)
                        T0v = TS0b(q).rearrange("p (vh cc) b -> p vh cc b", cc=dc)
                        v.wait_ge(sv, cv.n)
                        v.tensor_tensor(out=T0v, in0=u_b, in1=c2v_cv,
                                        op=A.subtract).then_inc(sv, 1)
                        cv.inc(1)
                        v.wait_ge(sv, cv.n)
                        v.tensor_tensor(out=AB, in0=TS0b(q), in1=gbch_b(CBLK),
                                        op=A.mult).then_inc(sv, 1)
                        cv.inc(1)
                        v.wait_ge(sp, snap["t4", t, c])
                        v.wait_ge(sv, cv.n)
                        v.tensor_tensor(out=v2c_c, in0=AB, in1=Cbb(q),
                                        op=A.add).then_inc(sv, 1)
                        cv.inc(1)
                    snap["vdve", t, c] = cv.n
                    if c == NCH_COL - 2:
                        snap["war_ab_sv", t + 1, 0] = cv.n
                    if c == NCH_COL - 1:
                        snap["war_ab_sv", t + 1, 1] = cv.n
                        snap["tsv_war", t + 1, 0] = cv.n

                @block.sync
                def _(sy, c=c, q=q, t=t):
                    sy.wait_ge(sv, snap["u", t, c])
                    sy.dma_start(
                        out=out_v[(t - 1) % OSL, :, c * VH_C:(c + 1) * VH_C, :],
                        in_=uslot_of(q),
                    ).then_inc(sos[q], 16)
                    cop[q].inc(16)
                    snap["odma", t, c] = cop[q].n
                    if t < T:
                        sy.wait_ge(sv, snap["vdve", t, c])
                        sy.dma_start(
                            out=v2c_hbm_v[:, c * CBLK:(c + 1) * CBLK, :],
                            in_=v2c[:, c * CBLK:(c + 1) * CBLK, :],
                        ).then_inc(svs[q], 16)
                        cvs[q].inc(16)

            for c in range(NCH_COL):
                emit_v_gather(c, t)
                if c >= 1:
                    emit_v_work(c - 1, t)
            emit_v_work(NCH_COL - 1, t)

            snap["v2c_staged", t] = (cvs[0].n, cvs[1].n)

        if adapter_only:
            @block.sync
            def _(sy):
                sy.wait_ge(sgam, 32)
                sy.dma_start(out=out_ext[0:1, 0:128, 0:128],
                             in_=gbch.ap().unsqueeze(0)).then_inc(svs[0], 16)
                cvs[0].inc(16)

        # drain: ensure all DMAs retired before block end
        @block.sync
        def _(sy):
            sy.wait_ge(sos[0], cop[0].n)
            sy.wait_ge(sos[1], cop[1].n)
            sy.wait_ge(shs[0], chp[0].n)
            sy.wait_ge(shs[1], chp[1].n)
            sy.wait_ge(svs[0], cvs[0].n)
            sy.wait_ge(svs[1], cvs[1].n)

    return nc


# ----------------------------------------------------------------------------
# host side
# ----------------------------------------------------------------------------

def _enum_maps(col_idx, row_idx, N, M, dc, dr):
    E = col_idx.shape[0]
    a_order = np.argsort(col_idx, kind="stable")     # a-pos -> edge
    r_order = np.argsort(row_idx, kind="stable")     # r-pos -> edge
    ap_ = np.arange(E)
    v = ap_ // dc
    ccol = ap_ % dc
    jA_of_apos = (v // 128) * (dc * 128) + ccol * 128 + (v % 128)
    m = ap_ // dr
    r = ap_ % dr
    jR_of_rpos = (m // 128) * (dr * 128) + r * 128 + (m % 128)
    jA_of_edge = np.empty(E, np.int64)
    jA_of_edge[a_order] = jA_of_apos
    jR_of_edge = np.empty(E, np.int64)
    jR_of_edge[r_order] = jR_of_rpos
    gA2R = np.empty(E, np.int64)
    gA2R[jR_of_edge] = jA_of_edge
    gR2A = np.empty(E, np.int64)
    gR2A[jA_of_edge] = jR_of_edge
    return gA2R.astype(np.int16), gR2A.astype(np.int16)


def _wrap16(idx):
    E = idx.shape[0]
    w = np.zeros((16, E // 16), np.int16)
    j = np.arange(E)
    w[j % 16, j // 16] = idx
    return np.tile(w, (8, 1))  # replicate for the 8 Q7 cores


_CACHE = {}
_LAST_IN_MAPS = None


def _get_graph(N, M, BS, T, dc, dr, NCH_ROW, NCH_COL, n_half):
    key = (N, M, BS, T, dc, dr, NCH_ROW, NCH_COL, n_half)
    if key not in _CACHE:
        nc = build_graph(N, M, BS, T, dc=dc, dr=dr, NCH_ROW=NCH_ROW,
                         NCH_COL=NCH_COL, n_half=n_half)
        nc.compile()
        _CACHE[key] = nc
    return _CACHE[key]


def kernel(**inputs):
    from concourse.bass_utils import run_bass_kernel_spmd

    chn_llr = np.asarray(inputs["chn_llr"], np.float32)
    W1 = np.asarray(inputs["W1"], np.float32)
    b1 = np.asarray(inputs["b1"], np.float32)
    W2 = np.asarray(inputs["W2"], np.float32)
    b2 = np.asarray(inputs["b2"], np.float32)
    col_idx = np.asarray(inputs["col_idx"])
    row_idx = np.asarray(inputs["row_idx"])
    N = int(inputs["n_var"])
    M = int(inputs["n_chk"])
    T = int(inputs["T"])
    B = chn_llr.shape[1]
    n_cores = 8
    BS = B // n_cores
    dc = col_idx.shape[0] // N
    dr = col_idx.shape[0] // M

    gA2R, gR2A = _enum_maps(np.asarray(col_idx), np.asarray(row_idx), N, M, dc, dr)
    ia2r = _wrap16(gA2R)
    ir2a = _wrap16(gR2A)
    import ml_dtypes
    w1t = np.ascontiguousarray(W1.T.astype(ml_dtypes.bfloat16))

    nc = _get_graph(N, M, 128, T, dc, dr, 2, 4, 2)

    in_maps = []
    for c in range(n_cores):
        shard = chn_llr[:, c * BS:(c + 1) * BS]           # [N, BS]
        chn_pm = np.ascontiguousarray(
            shard.reshape(N // 128, 128, BS).transpose(1, 0, 2).reshape(128, -1))
        in_maps.append({
            "chn": chn_pm,
            "w1t": w1t,
            "b1": b1.reshape(1, N),
            "w2": W2,
            "b2": b2.reshape(1, 3),
            "idx_a2r": ia2r,
            "idx_r2a": ir2a,
        })

    global _LAST_IN_MAPS
    _LAST_IN_MAPS = in_maps
    res = run_bass_kernel_spmd(nc, in_maps, core_ids=list(range(n_cores)))
    outs = [np.asarray(r["out"].astype(np.float32)).reshape(T, N, BS)
            for r in res.results]
    return np.concatenate(outs, axis=2)
